# revision 1
# baseline (speedup 1.0000x reference)
"""Trainium2 Bass kernel for BilinearAttention GNN message passing.

Math (see reference):
  q = (x @ nonneg(q_w).T) / D ; k = (x @ nonneg(k_w).T) / D
  ego = q*q*nonneg(ego_scale)
  G[i,h] = sum_j mask[i,j] * k[src[i,j], h]          (dst grouped per cell)
  sum_local = q * G / (actual_k + 1e-6)
  s = ego + sum_local ; attn = s / (sum_h s + 1e-9)
  res = attn @ nonneg(v_w).T + nonneg(bias)

Distribution / algorithm (v2):
  Cells are sharded over 8 cores (12500 real -> 12544 padded each). Each
  core computes q and k^T for its cells, AllGathers k^T so that SBUF
  partition p holds column h=p%16 of source-core p//16's k table
  ("sections"). Valid edges are compacted on the host, bucketed per
  (source-section group, 1568-cell sub-pass), sorted by target cell, and
  gathered with gpsimd.ap_gather (all 8 Q7 cores in parallel, one 16-
  partition group per section). Per-cell sums come from a masked-reset
  prefix scan (state = m*state + kv, m=0 at run starts) followed by a
  gpsimd.local_scatter that drops each run-end value into the cell's
  column of G. A [128,16] block-indicator matmul reduces the 8 section
  groups, and the attention math + final matmul run on [16, cells]
  grids with the output produced transposed ([512, C] bf16).
"""

import sys

sys.path.insert(0, "/opt/trn_rl_repo")

import numpy as np

import concourse.bacc as bacc
import concourse.bass as bass
import concourse.mybir as mybir
import concourse.tile as tile
from concourse.masks import make_identity

P = 128


class Cfg:
    def __init__(self, N=100000, D=512, H=16, K=32, NC=8, W=None):
        assert N % NC == 0
        self.N, self.D, self.H, self.K, self.NC = N, D, H, K, NC
        self.nr = N // NC                  # real cells per core
        self.T = -(-self.nr // P)          # 128-row tiles per core
        self.C = self.T * P                # padded cells per core
        self.NE = self.C + 1               # table cols (zero col 0)
        self.NPASS = 8
        assert self.C % self.NPASS == 0
        self.CELLS = self.C // self.NPASS  # cells per sub-pass
        self.DCH = D // P
        self.W = W                         # gathered idx per (group, pass)
        self.CCW = 512                     # phase-1 cell chunk


def build(cfg: Cfg):
    f32, bf16, i16 = mybir.dt.float32, mybir.dt.bfloat16, mybir.dt.int16
    nc = bacc.Bacc("TRN2", target_bir_lowering=False, debug=False,
                   enable_asserts=False, num_devices=cfg.NC)
    D, H, C, NE, W = cfg.D, cfg.H, cfg.C, cfg.NE, cfg.W
    NPASS, CELLS, DCH = cfg.NPASS, cfg.CELLS, cfg.DCH

    xT_d = nc.dram_tensor("xT", [D, C], bf16, kind="ExternalInput")
    qkw_d = nc.dram_tensor("qkw", [2 * H, D], f32, kind="ExternalInput")
    vw_d = nc.dram_tensor("vw", [D, H], f32, kind="ExternalInput")
    ego_d = nc.dram_tensor("ego", [1, H], f32, kind="ExternalInput")
    bias_d = nc.dram_tensor("bias", [1, D], f32, kind="ExternalInput")
    idx_d = nc.dram_tensor("idx", [NPASS, P, W // 16], i16, kind="ExternalInput")
    msk_d = nc.dram_tensor("msk", [NPASS, P, W], bf16, kind="ExternalInput")
    emap_d = nc.dram_tensor("emap", [NPASS, P, W], i16, kind="ExternalInput")
    invk_d = nc.dram_tensor("invk", [H, C], f32, kind="ExternalInput")
    r16_d = nc.dram_tensor("r16", [P, H], bf16, kind="ExternalInput")

    qk_sh = nc.dram_tensor("qk_sh", [2 * H, C], f32, kind="Internal")
    kfullT = nc.dram_tensor("kfullT", [P, C], f32, kind="Internal",
                            addr_space="Shared")
    resT = nc.dram_tensor("resT", [D, C], bf16, kind="ExternalOutput")

    SUB = 392  # matmul free-dim sub-chunk inside a 1568-cell chunk

    with tile.TileContext(nc) as tc:
        with (
            tc.tile_pool(name="const", bufs=1) as cpool,
            tc.tile_pool(name="x_stream", bufs=3) as xpool,
            tc.tile_pool(name="stage", bufs=1) as stpool,
            tc.tile_pool(name="gidx", bufs=2) as gip,
            tc.tile_pool(name="gmsk", bufs=2) as gmp,
            tc.tile_pool(name="gemap", bufs=2) as gep,
            tc.tile_pool(name="kv", bufs=2) as kvp,
            tc.tile_pool(name="sbf", bufs=2) as sbp,
            tc.tile_pool(name="gtab", bufs=8) as gtp,
            tc.tile_pool(name="attn", bufs=1) as apool,
            tc.tile_pool(name="subs", bufs=2) as subp,
            tc.tile_pool(name="ps_qk", bufs=2, space="PSUM") as ps_qk,
            tc.tile_pool(name="ps_t", bufs=2, space="PSUM") as ps_t,
            tc.tile_pool(name="ps_gn", bufs=2, space="PSUM") as ps_gn,
            tc.tile_pool(name="ps_r", bufs=2, space="PSUM") as ps_r,
        ):
            ident = cpool.tile([P, P], f32)
            make_identity(nc, ident[:])

            def nonneg(dst, src, tmp):
                # elu(w)+1 = exp(min(w,0)) + max(w,0)
                nc.vector.tensor_scalar_min(tmp[:], src[:], 0.0)
                nc.scalar.activation(tmp[:], tmp[:],
                                     mybir.ActivationFunctionType.Exp)
                nc.vector.tensor_scalar_max(dst[:], src[:], 0.0)
                nc.vector.tensor_add(dst[:], dst[:], tmp[:])

            # ---- weight prep (temporaries live in the stage pool) ----
            qkw_sb = stpool.tile([2 * H, D], f32, tag="w_src")
            nc.sync.dma_start(out=qkw_sb[:], in_=qkw_d[:, :])
            qkw_nn = stpool.tile([2 * H, D], f32, tag="w_nn")
            tmp_a = stpool.tile([2 * H, D], f32, tag="w_tmp")
            nonneg(qkw_nn, qkw_sb, tmp_a)
            nc.vector.tensor_scalar_mul(qkw_nn[:], qkw_nn[:], 1.0 / D)
            qkwT_bf = cpool.tile([P, DCH, 2 * H], bf16)
            for dc in range(DCH):
                pt = ps_t.tile([P, P], f32, tag="tp")
                nc.tensor.transpose(out=pt[:, 0:2 * H],
                                    in_=qkw_nn[:, dc * P:(dc + 1) * P],
                                    identity=ident[:2 * H, :2 * H])
                nc.vector.tensor_copy(qkwT_bf[:, dc, :], pt[:, 0:2 * H])

            vwT = stpool.tile([H, D], f32, tag="w_src")
            for dc in range(DCH):
                vc = stpool.tile([P, H], f32, tag="vld")
                nc.sync.dma_start(out=vc[:], in_=vw_d[dc * P:(dc + 1) * P, :])
                pt2 = ps_t.tile([P, P], f32, tag="tp")
                nc.tensor.transpose(out=pt2[0:H, :], in_=vc[:], identity=ident[:])
                nc.vector.tensor_copy(vwT[:, dc * P:(dc + 1) * P], pt2[0:H, :])
            vwT_nn = stpool.tile([H, D], f32, tag="w_nn2")
            tmp_v = stpool.tile([H, D], f32, tag="w_tmp")
            nonneg(vwT_nn, vwT, tmp_v)
            vwT_bf = cpool.tile([H, D], bf16)
            nc.vector.tensor_copy(vwT_bf[:], vwT_nn[:])

            bias_sb = stpool.tile([1, D], f32, tag="w_src")
            nc.sync.dma_start(out=bias_sb[:], in_=bias_d[:, :])
            bias_nn = stpool.tile([1, D], f32, tag="w_nn")
            tmp_b = stpool.tile([1, D], f32, tag="w_tmp")
            nonneg(bias_nn, bias_sb, tmp_b)
            bias_bf = cpool.tile([1, D], bf16)
            nc.vector.tensor_copy(bias_bf[:], bias_nn[:])

            ego_sb = stpool.tile([1, H], f32, tag="w_src")
            nc.sync.dma_start(out=ego_sb[:], in_=ego_d[:, :])
            ego_nn = stpool.tile([1, H], f32, tag="w_nn")
            tmp_g = stpool.tile([1, H], f32, tag="w_tmp")
            nonneg(ego_nn, ego_sb, tmp_g)
            pe = ps_t.tile([P, P], f32, tag="tp")
            nc.tensor.transpose(out=pe[0:H, 0:1], in_=ego_nn[:],
                                identity=ident[:1, :1])
            ego_rt = cpool.tile([H, 1], f32)
            nc.scalar.activation(ego_rt[:], pe[0:H, 0:1],
                                 mybir.ActivationFunctionType.Sqrt)

            ones16 = cpool.tile([H, H], f32)
            nc.vector.memset(ones16[:], 1.0)
            eps16 = cpool.tile([H, 1], f32)
            nc.vector.memset(eps16[:], 1e-9)
            ones1 = cpool.tile([1, SUB], bf16)
            nc.vector.memset(ones1[:], 1.0)
            r16 = cpool.tile([P, H], bf16)
            nc.sync.dma_start(out=r16[:], in_=r16_d[:, :])

            # ---- phase 1a: k embeddings only (critical path to AllGather) ----
            def qk_half(c0, cw, lo, hi, dst_rows):
                xt = xpool.tile([P, DCH, cfg.CCW], bf16, tag="xt")
                nc.sync.dma_start(
                    out=xt[:, :, :cw],
                    in_=xT_d[:, c0:c0 + cw].rearrange("(dc p) c -> p dc c", p=P))
                pqk = ps_qk.tile([H, cfg.CCW], f32, tag="pqk")
                for dc in range(DCH):
                    nc.tensor.matmul(pqk[:, :cw], lhsT=qkwT_bf[:, dc, lo:hi],
                                     rhs=xt[:, dc, :cw],
                                     start=(dc == 0), stop=(dc == DCH - 1))
                stg = stpool.tile([H, cfg.CCW], f32, tag="stg")
                nc.vector.tensor_copy(stg[:, :cw], pqk[:, :cw])
                nc.sync.dma_start(out=qk_sh[dst_rows, c0:c0 + cw],
                                  in_=stg[:, :cw])

            for c0 in range(0, C, cfg.CCW):
                cw = min(cfg.CCW, C - c0)
                qk_half(c0, cw, H, 2 * H, slice(H, 2 * H))

            # ---- AllGather k^T ----
            nc.gpsimd.collective_compute(
                "AllGather", mybir.AluOpType.bypass,
                replica_groups=[list(range(cfg.NC))],
                ins=[qk_sh[H:2 * H, :].opt()],
                outs=[kfullT[:, :].opt()],
            )

            table = cpool.tile([P, NE], f32)
            nc.vector.memset(table[:, 0:1], 0.0)
            nc.sync.dma_start(out=table[:, 1:NE], in_=kfullT[:, :])

            # gather-input prefetch (2 passes ahead)
            def load_gather_inputs(ps):
                idxt = gip.tile([P, W // 16], i16, tag="idx")
                nc.sync.dma_start(out=idxt[:], in_=idx_d[ps, :, :])
                mskt = gmp.tile([P, W], bf16, tag="msk")
                nc.sync.dma_start(out=mskt[:], in_=msk_d[ps, :, :])
                emapt = gep.tile([P, W], i16, tag="emap")
                nc.sync.dma_start(out=emapt[:], in_=emap_d[ps, :, :])
                return idxt, mskt, emapt

            pre = {0: load_gather_inputs(0), 1: load_gather_inputs(1)}

            # ---- phase 1b: q embeddings (overlaps the gather passes) ----
            for c0 in range(0, C, cfg.CCW):
                cw = min(cfg.CCW, C - c0)
                qk_half(c0, cw, 0, H, slice(0, H))

            # ---- attention + output for one 1568-cell chunk ----
            def attn_chunk(ch, Gt):
                r0 = ch * CELLS
                q_t = apool.tile([H, CELLS], f32, tag="q")
                nc.sync.dma_start(out=q_t[:], in_=qk_sh[0:H, r0:r0 + CELLS])
                ik_t = apool.tile([H, CELLS], f32, tag="ik")
                nc.sync.dma_start(out=ik_t[:], in_=invk_d[:, r0:r0 + CELLS])

                nc.vector.tensor_tensor(out=ik_t[:], in0=q_t[:], in1=ik_t[:],
                                        op=mybir.AluOpType.mult)
                egt = apool.tile([H, CELLS], f32, tag="egt")
                nc.scalar.activation(egt[:], q_t[:],
                                     mybir.ActivationFunctionType.Square,
                                     scale=ego_rt[:])

                for s0 in range(0, CELLS, SUB):
                    sl = slice(s0, s0 + SUB)
                    pg = ps_gn.tile([H, SUB], f32, tag="gn")
                    nc.tensor.matmul(pg[:], lhsT=r16[:], rhs=Gt[:, sl],
                                     start=True, stop=True)
                    # DVE reads PSUM ~6x slower than SBUF; stage via ACT
                    g16s = subp.tile([H, SUB], f32, tag="g16")
                    nc.scalar.activation(g16s[:], pg[:],
                                         mybir.ActivationFunctionType.Copy)
                    sss = subp.tile([H, SUB], f32, tag="ss")
                    nc.vector.tensor_tensor(out=sss[:], in0=g16s[:],
                                            in1=ik_t[:, sl],
                                            op=mybir.AluOpType.mult)
                    nc.vector.tensor_tensor(out=sss[:], in0=sss[:],
                                            in1=egt[:, sl],
                                            op=mybir.AluOpType.add)
                    pn = ps_gn.tile([H, SUB], f32, tag="gn")
                    nc.tensor.matmul(pn[:], lhsT=ones16[:], rhs=sss[:],
                                     start=True, stop=True)
                    nrms = subp.tile([H, SUB], f32, tag="nrm")
                    nc.scalar.activation(nrms[:], pn[:],
                                         mybir.ActivationFunctionType.Identity,
                                         bias=eps16[:])
                    nc.vector.reciprocal(nrms[:], nrms[:])
                    ats = subp.tile([H, SUB], bf16, tag="attn")
                    nc.vector.tensor_tensor(out=ats[:], in0=sss[:], in1=nrms[:],
                                            op=mybir.AluOpType.mult)
                    for dc in range(DCH):
                        pr = ps_r.tile([P, SUB], f32, tag="pr")
                        nc.tensor.matmul(pr[:], lhsT=vwT_bf[:, dc * P:(dc + 1) * P],
                                         rhs=ats[:], start=True, stop=False)
                        nc.tensor.matmul(pr[:], lhsT=bias_bf[:, dc * P:(dc + 1) * P],
                                         rhs=ones1[:], start=False, stop=True)
                        rsb = subp.tile([P, SUB], bf16, tag="rsb")
                        nc.scalar.activation(rsb[:], pr[:],
                                             mybir.ActivationFunctionType.Copy)
                        nc.sync.dma_start(
                            out=resT[dc * P:(dc + 1) * P, r0 + s0:r0 + s0 + SUB],
                            in_=rsb[:])

            # ---- phases 3+4: gathers with attention one pass behind ----
            Gts = {}
            for ps in range(NPASS):
                idxt, mskt, emapt = pre.pop(ps)
                if ps + 2 < NPASS:
                    pre[ps + 2] = load_gather_inputs(ps + 2)

                kv = kvp.tile([P, W], f32, tag="kv")
                nc.gpsimd.ap_gather(kv[:], table[:], idxt[:],
                                    channels=P, num_elems=NE, d=1, num_idxs=W)
                sbf = sbp.tile([P, W], bf16, tag="sbf")
                nc.vector.tensor_tensor_scan(
                    sbf[:], mskt[:], kv[:], 0.0,
                    op0=mybir.AluOpType.mult, op1=mybir.AluOpType.add)
                # local_scatter zero-fills and fully overwrites dst
                Gt = gtp.tile([P, CELLS], bf16, tag="g")
                nc.gpsimd.local_scatter(Gt[:], sbf[:], emapt[:],
                                        channels=P, num_elems=CELLS, num_idxs=W)
                Gts[ps] = Gt
                if ps >= 1:
                    attn_chunk(ps - 1, Gts.pop(ps - 1))
            attn_chunk(NPASS - 1, Gts.pop(NPASS - 1))

    nc.compile()
    return nc


def prep_inputs(cfg: Cfg, x, adj_list, q_w, k_w, v_w, ego_scale, bias):
    """Host-side sharding/index prep.

    Returns (list of per-core input dicts, W). Sets cfg.W if unset.
    """
    import ml_dtypes
    N, D, H, K, NC = cfg.N, cfg.D, cfg.H, cfg.K, cfg.NC
    nr, C, NPASS, CELLS = cfg.nr, cfg.C, cfg.NPASS, cfg.CELLS

    src = np.asarray(adj_list[0], dtype=np.int64)
    dst = np.asarray(adj_list[1], dtype=np.int64)
    mk = np.asarray(adj_list[2]) != 0

    e_src, e_dst = src[mk], dst[mk]
    core = e_dst // nr
    lcell = e_dst % nr
    group = e_src // nr
    lsrc = (e_src % nr + 1).astype(np.int64)

    # balance cells across the 8 sub-passes to minimize the padded gather
    # width W = max over (core, group, pass) segment size
    cnt = np.zeros((NC, nr, 8), np.int32)
    np.add.at(cnt, (core, lcell, group), 1)
    perm = np.empty((NC, nr), np.int64)    # real cell -> column position
    for c in range(NC):
        tot = cnt[c].sum(1)
        order_c = np.argsort(-tot, kind="stable")
        loads = np.zeros((8, NPASS), np.int64)
        space = np.full(NPASS, CELLS)
        nxt = np.arange(NPASS) * CELLS
        for cell in order_c:
            v = cnt[c, cell][:, None]
            cand = (loads + v).max(0).astype(np.float64)
            cand[space == 0] = np.inf
            p = int(np.argmin(cand))
            loads[:, p] += cnt[c, cell]
            space[p] -= 1
            perm[c, cell] = nxt[p]
            nxt[p] += 1

    pos = perm[core, lcell]
    pss = pos // CELLS
    cell_in_ps = pos % CELLS

    key = ((core * 8 + group) * NPASS + pss) * 16384 + pos
    order = np.argsort(key, kind="stable")
    sc = core[order]
    sg = group[order]
    sp_ = pss[order]
    scell = cell_in_ps[order]
    ssrc = lsrc[order]

    seg = (sc * 8 + sg) * NPASS + sp_
    nseg = NC * 8 * NPASS
    seg_counts = np.bincount(seg, minlength=nseg)
    wmax = int(seg_counts.max())
    if cfg.W is None:
        cfg.W = max(16, -(-wmax // 16) * 16)
    W = cfg.W
    assert wmax <= W, f"W={W} too small for max segment {wmax}"
    seg_start = np.concatenate([[0], np.cumsum(seg_counts)])[:-1]
    pos = np.arange(len(order)) - seg_start[seg]

    same_prev = np.empty(len(order), dtype=bool)
    same_prev[0] = False
    same_prev[1:] = (seg[1:] == seg[:-1]) & (scell[1:] == scell[:-1])
    is_start = ~same_prev
    same_next = np.empty(len(order), dtype=bool)
    same_next[-1] = False
    same_next[:-1] = same_prev[1:]
    is_end = ~same_next

    idx8 = np.zeros((NC, NPASS, 8, W), np.int16)
    idx8[sc, sp_, sg, pos] = ssrc.astype(np.int16)
    msk8 = np.ones((NC, NPASS, 8, W), np.float32)
    msk8[sc[is_start], sp_[is_start], sg[is_start], pos[is_start]] = 0.0
    emap8 = np.full((NC, NPASS, 8, W), -1, np.int16)
    emap8[sc[is_end], sp_[is_end], sg[is_end], pos[is_end]] = \
        scell[is_end].astype(np.int16)

    # wrap idx streams: stream pos j -> (partition 16g + j%16, col j//16)
    idx_np = idx8.reshape(NC, NPASS, 8, W // 16, 16).transpose(0, 1, 2, 4, 3) \
        .reshape(NC, NPASS, P, W // 16)
    msk_np = np.repeat(msk8, 16, axis=2).astype(ml_dtypes.bfloat16)
    emap_np = np.repeat(emap8, 16, axis=2)

    ak = mk.reshape(N, K).sum(axis=1).astype(np.float32)
    inv_ak = (1.0 / (ak + 1e-6)).astype(np.float32)

    qkw_np = np.concatenate([np.asarray(q_w), np.asarray(k_w)],
                            axis=0).astype(np.float32)
    r16_np = (np.arange(P)[:, None] % 16 == np.arange(H)[None, :]) \
        .astype(ml_dtypes.bfloat16)

    in_maps = []
    for c in range(NC):
        xs = np.zeros((C, D), dtype=np.float32)
        xs[perm[c]] = np.asarray(x[c * nr:(c + 1) * nr], dtype=np.float32)
        xT = np.ascontiguousarray(xs.T).astype(ml_dtypes.bfloat16)

        ik = np.full((C,), 1e6, dtype=np.float32)
        ik[perm[c]] = inv_ak[c * nr:(c + 1) * nr]
        invk_np = np.broadcast_to(ik[None, :], (H, C)).copy()

        in_maps.append({
            "xT": xT,
            "qkw": qkw_np,
            "vw": np.asarray(v_w, dtype=np.float32),
            "ego": np.asarray(ego_scale, dtype=np.float32),
            "bias": np.asarray(bias, dtype=np.float32),
            "idx": idx_np[c],
            "msk": msk_np[c],
            "emap": emap_np[c],
            "invk": invk_np,
            "r16": r16_np,
        })
    return in_maps, W, perm


_CACHE = {}


def _get_compiled(cfg: Cfg):
    key = (cfg.N, cfg.D, cfg.H, cfg.K, cfg.NC, cfg.W)
    if key not in _CACHE:
        _CACHE[key] = build(cfg)
    return _CACHE[key]


def kernel(x, adj_list, q_w, k_w, v_w, ego_scale, bias, _trace=False):
    import concourse.bass_utils as bass_utils
    x = np.asarray(x)
    adj_list = np.asarray(adj_list)
    N, D = x.shape
    H = np.asarray(q_w).shape[0]
    K = adj_list.shape[1] // N
    cfg = Cfg(N=N, D=D, H=H, K=K, NC=8)

    dst = np.asarray(adj_list[1], dtype=np.int64)
    assert np.array_equal(dst, np.repeat(np.arange(N, dtype=np.int64), K)), \
        "kernel requires adj_list[1] grouped per target cell"

    in_maps, _, perm = prep_inputs(cfg, x, adj_list, q_w, k_w, v_w,
                                   ego_scale, bias)
    nc = _get_compiled(cfg)
    r = bass_utils.run_bass_kernel_spmd(nc, in_maps, core_ids=list(range(cfg.NC)),
                                        trace=_trace)
    out = np.concatenate(
        [r.results[c]["resT"].astype(np.float32).T[perm[c]]
         for c in range(cfg.NC)], axis=0)
    if _trace:
        return out, r
    return out



# revision 2
# speedup vs baseline: 1.0253x; 1.0253x over previous
"""Trainium2 Bass kernel for BilinearAttention GNN message passing.

Math (see reference):
  q = (x @ nonneg(q_w).T) / D ; k = (x @ nonneg(k_w).T) / D
  ego = q*q*nonneg(ego_scale)
  G[i,h] = sum_j mask[i,j] * k[src[i,j], h]          (dst grouped per cell)
  sum_local = q * G / (actual_k + 1e-6)
  s = ego + sum_local ; attn = s / (sum_h s + 1e-9)
  res = attn @ nonneg(v_w).T + nonneg(bias)

Distribution / algorithm (v2):
  Cells are sharded over 8 cores (12500 real -> 12544 padded each). Each
  core computes q and k^T for its cells, AllGathers k^T so that SBUF
  partition p holds column h=p%16 of source-core p//16's k table
  ("sections"). Valid edges are compacted on the host, bucketed per
  (source-section group, 1568-cell sub-pass), sorted by target cell, and
  gathered with gpsimd.ap_gather (all 8 Q7 cores in parallel, one 16-
  partition group per section). Per-cell sums come from a masked-reset
  prefix scan (state = m*state + kv, m=0 at run starts) followed by a
  gpsimd.local_scatter that drops each run-end value into the cell's
  column of G. A [128,16] block-indicator matmul reduces the 8 section
  groups, and the attention math + final matmul run on [16, cells]
  grids with the output produced transposed ([512, C] bf16).
"""

import sys

sys.path.insert(0, "/opt/trn_rl_repo")

import numpy as np

import concourse.bacc as bacc
import concourse.bass as bass
import concourse.mybir as mybir
import concourse.tile as tile
from concourse.masks import make_identity

P = 128


class Cfg:
    def __init__(self, N=100000, D=512, H=16, K=32, NC=8, W=None):
        assert N % NC == 0
        self.N, self.D, self.H, self.K, self.NC = N, D, H, K, NC
        self.nr = N // NC                  # real cells per core
        self.T = -(-self.nr // P)          # 128-row tiles per core
        self.C = self.T * P                # padded cells per core
        self.NE = self.C + 1               # table cols (zero col 0)
        self.NPASS = 8
        assert self.C % self.NPASS == 0
        self.CELLS = self.C // self.NPASS  # cells per sub-pass
        self.DCH = D // P
        self.W = W                         # gathered idx per (group, pass)
        self.CCW = 512                     # phase-1 cell chunk


def build(cfg: Cfg):
    f32, bf16, i16 = mybir.dt.float32, mybir.dt.bfloat16, mybir.dt.int16
    nc = bacc.Bacc("TRN2", target_bir_lowering=False, debug=False,
                   enable_asserts=False, num_devices=cfg.NC)
    D, H, C, NE, W = cfg.D, cfg.H, cfg.C, cfg.NE, cfg.W
    NPASS, CELLS, DCH = cfg.NPASS, cfg.CELLS, cfg.DCH

    xT_d = nc.dram_tensor("xT", [D, C], bf16, kind="ExternalInput")
    qkw_d = nc.dram_tensor("qkw", [2 * H, D], f32, kind="ExternalInput")
    vw_d = nc.dram_tensor("vw", [D, H], f32, kind="ExternalInput")
    ego_d = nc.dram_tensor("ego", [1, H], f32, kind="ExternalInput")
    bias_d = nc.dram_tensor("bias", [1, D], f32, kind="ExternalInput")
    idx_d = nc.dram_tensor("idx", [NPASS, P, W // 16], i16, kind="ExternalInput")
    msk_d = nc.dram_tensor("msk", [NPASS, P, W], bf16, kind="ExternalInput")
    emap_d = nc.dram_tensor("emap", [NPASS, P, W], i16, kind="ExternalInput")
    invk_d = nc.dram_tensor("invk", [H, C], f32, kind="ExternalInput")
    r16_d = nc.dram_tensor("r16", [P, H], bf16, kind="ExternalInput")

    qk_sh = nc.dram_tensor("qk_sh", [2 * H, C], f32, kind="Internal")
    kfullT = nc.dram_tensor("kfullT", [P, C], f32, kind="Internal",
                            addr_space="Shared")
    resT = nc.dram_tensor("resT", [D, C], bf16, kind="ExternalOutput")

    SUB = 392  # matmul free-dim sub-chunk inside a 1568-cell chunk

    with tile.TileContext(nc) as tc:
        with (
            tc.tile_pool(name="const", bufs=1) as cpool,
            tc.tile_pool(name="x_stream", bufs=3) as xpool,
            tc.tile_pool(name="stage", bufs=1) as stpool,
            tc.tile_pool(name="gidx", bufs=2) as gip,
            tc.tile_pool(name="gmsk", bufs=2) as gmp,
            tc.tile_pool(name="gemap", bufs=2) as gep,
            tc.tile_pool(name="kv", bufs=2) as kvp,
            tc.tile_pool(name="sbf", bufs=2) as sbp,
            tc.tile_pool(name="gtab", bufs=8) as gtp,
            tc.tile_pool(name="attn", bufs=1) as apool,
            tc.tile_pool(name="subs", bufs=2) as subp,
            tc.tile_pool(name="ps_qk", bufs=2, space="PSUM") as ps_qk,
            tc.tile_pool(name="ps_t", bufs=2, space="PSUM") as ps_t,
            tc.tile_pool(name="ps_gn", bufs=2, space="PSUM") as ps_gn,
            tc.tile_pool(name="ps_r", bufs=2, space="PSUM") as ps_r,
        ):
            ident = cpool.tile([P, P], f32)
            make_identity(nc, ident[:])

            def nonneg(dst, src, tmp):
                # elu(w)+1 = exp(min(w,0)) + max(w,0)
                nc.vector.tensor_scalar_min(tmp[:], src[:], 0.0)
                nc.scalar.activation(tmp[:], tmp[:],
                                     mybir.ActivationFunctionType.Exp)
                nc.vector.tensor_scalar_max(dst[:], src[:], 0.0)
                nc.vector.tensor_add(dst[:], dst[:], tmp[:])

            # ---- weight prep (temporaries live in the stage pool) ----
            qkw_sb = stpool.tile([2 * H, D], f32, tag="w_src")
            nc.sync.dma_start(out=qkw_sb[:], in_=qkw_d[:, :])
            qkw_nn = stpool.tile([2 * H, D], f32, tag="w_nn")
            tmp_a = stpool.tile([2 * H, D], f32, tag="w_tmp")
            nonneg(qkw_nn, qkw_sb, tmp_a)
            nc.vector.tensor_scalar_mul(qkw_nn[:], qkw_nn[:], 1.0 / D)
            qkwT_bf = cpool.tile([P, DCH, 2 * H], bf16)
            for dc in range(DCH):
                pt = ps_t.tile([P, P], f32, tag="tp")
                nc.tensor.transpose(out=pt[:, 0:2 * H],
                                    in_=qkw_nn[:, dc * P:(dc + 1) * P],
                                    identity=ident[:2 * H, :2 * H])
                nc.vector.tensor_copy(qkwT_bf[:, dc, :], pt[:, 0:2 * H])

            vwT = stpool.tile([H, D], f32, tag="w_src")
            for dc in range(DCH):
                vc = stpool.tile([P, H], f32, tag="vld")
                nc.sync.dma_start(out=vc[:], in_=vw_d[dc * P:(dc + 1) * P, :])
                pt2 = ps_t.tile([P, P], f32, tag="tp")
                nc.tensor.transpose(out=pt2[0:H, :], in_=vc[:], identity=ident[:])
                nc.vector.tensor_copy(vwT[:, dc * P:(dc + 1) * P], pt2[0:H, :])
            vwT_nn = stpool.tile([H, D], f32, tag="w_nn2")
            tmp_v = stpool.tile([H, D], f32, tag="w_tmp")
            nonneg(vwT_nn, vwT, tmp_v)
            vwT_bf = cpool.tile([H, D], bf16)
            nc.vector.tensor_copy(vwT_bf[:], vwT_nn[:])

            bias_sb = stpool.tile([1, D], f32, tag="w_src")
            nc.sync.dma_start(out=bias_sb[:], in_=bias_d[:, :])
            bias_nn = stpool.tile([1, D], f32, tag="w_nn")
            tmp_b = stpool.tile([1, D], f32, tag="w_tmp")
            nonneg(bias_nn, bias_sb, tmp_b)
            # per-partition bias column per d-chunk (consumed as activation
            # bias on the output copy; replaces the 1-contraction matmuls)
            biasT = cpool.tile([P, DCH], f32)
            for dc in range(DCH):
                ptb = ps_t.tile([P, P], f32, tag="tp")
                nc.tensor.transpose(out=ptb[:, 0:1],
                                    in_=bias_nn[:, dc * P:(dc + 1) * P],
                                    identity=ident[:1, :1])
                nc.vector.tensor_copy(biasT[:, dc:dc + 1], ptb[:, 0:1])

            ego_sb = stpool.tile([1, H], f32, tag="w_src")
            nc.sync.dma_start(out=ego_sb[:], in_=ego_d[:, :])
            ego_nn = stpool.tile([1, H], f32, tag="w_nn")
            tmp_g = stpool.tile([1, H], f32, tag="w_tmp")
            nonneg(ego_nn, ego_sb, tmp_g)
            pe = ps_t.tile([P, P], f32, tag="tp")
            nc.tensor.transpose(out=pe[0:H, 0:1], in_=ego_nn[:],
                                identity=ident[:1, :1])
            ego_rt = cpool.tile([H, 1], f32)
            nc.scalar.activation(ego_rt[:], pe[0:H, 0:1],
                                 mybir.ActivationFunctionType.Sqrt)

            ones16 = cpool.tile([H, H], f32)
            nc.vector.memset(ones16[:], 1.0)
            eps16 = cpool.tile([H, 1], f32)
            nc.vector.memset(eps16[:], 1e-9)
            r16 = cpool.tile([P, H], bf16)
            nc.sync.dma_start(out=r16[:], in_=r16_d[:, :])

            # ---- phase 1a: k embeddings only (critical path to AllGather) ----
            def qk_half(c0, cw, lo, hi, dst_rows):
                xt = xpool.tile([P, DCH, cfg.CCW], bf16, tag="xt")
                nc.sync.dma_start(
                    out=xt[:, :, :cw],
                    in_=xT_d[:, c0:c0 + cw].rearrange("(dc p) c -> p dc c", p=P))
                pqk = ps_qk.tile([H, cfg.CCW], f32, tag="pqk")
                for dc in range(DCH):
                    nc.tensor.matmul(pqk[:, :cw], lhsT=qkwT_bf[:, dc, lo:hi],
                                     rhs=xt[:, dc, :cw],
                                     start=(dc == 0), stop=(dc == DCH - 1))
                stg = stpool.tile([H, cfg.CCW], f32, tag="stg")
                nc.vector.tensor_copy(stg[:, :cw], pqk[:, :cw])
                nc.sync.dma_start(out=qk_sh[dst_rows, c0:c0 + cw],
                                  in_=stg[:, :cw])

            for c0 in range(0, C, cfg.CCW):
                cw = min(cfg.CCW, C - c0)
                qk_half(c0, cw, H, 2 * H, slice(H, 2 * H))

            # ---- AllGather k^T ----
            nc.gpsimd.collective_compute(
                "AllGather", mybir.AluOpType.bypass,
                replica_groups=[list(range(cfg.NC))],
                ins=[qk_sh[H:2 * H, :].opt()],
                outs=[kfullT[:, :].opt()],
            )

            table = cpool.tile([P, NE], f32)
            nc.vector.memset(table[:, 0:1], 0.0)
            nc.sync.dma_start(out=table[:, 1:NE], in_=kfullT[:, :])

            # gather-input prefetch (2 passes ahead)
            def load_gather_inputs(ps):
                idxt = gip.tile([P, W // 16], i16, tag="idx")
                nc.sync.dma_start(out=idxt[:], in_=idx_d[ps, :, :])
                mskt = gmp.tile([P, W], bf16, tag="msk")
                nc.sync.dma_start(out=mskt[:], in_=msk_d[ps, :, :])
                emapt = gep.tile([P, W], i16, tag="emap")
                nc.sync.dma_start(out=emapt[:], in_=emap_d[ps, :, :])
                return idxt, mskt, emapt

            pre = {0: load_gather_inputs(0), 1: load_gather_inputs(1)}

            # ---- phase 1b: q embeddings (overlaps the gather passes) ----
            for c0 in range(0, C, cfg.CCW):
                cw = min(cfg.CCW, C - c0)
                qk_half(c0, cw, 0, H, slice(0, H))

            # ---- attention + output for one 1568-cell chunk ----
            def attn_chunk(ch, Gt):
                r0 = ch * CELLS
                q_t = apool.tile([H, CELLS], f32, tag="q")
                nc.sync.dma_start(out=q_t[:], in_=qk_sh[0:H, r0:r0 + CELLS])
                ik_t = apool.tile([H, CELLS], f32, tag="ik")
                nc.sync.dma_start(out=ik_t[:], in_=invk_d[:, r0:r0 + CELLS])

                nc.vector.tensor_tensor(out=ik_t[:], in0=q_t[:], in1=ik_t[:],
                                        op=mybir.AluOpType.mult)
                egt = apool.tile([H, CELLS], f32, tag="egt")
                nc.scalar.activation(egt[:], q_t[:],
                                     mybir.ActivationFunctionType.Square,
                                     scale=ego_rt[:])

                for s0 in range(0, CELLS, SUB):
                    sl = slice(s0, s0 + SUB)
                    pg = ps_gn.tile([H, SUB], f32, tag="gn")
                    nc.tensor.matmul(pg[:], lhsT=r16[:], rhs=Gt[:, sl],
                                     start=True, stop=True)
                    # DVE reads PSUM ~6x slower than SBUF; stage via ACT
                    g16s = subp.tile([H, SUB], f32, tag="g16")
                    nc.scalar.activation(g16s[:], pg[:],
                                         mybir.ActivationFunctionType.Copy)
                    sss = subp.tile([H, SUB], f32, tag="ss")
                    nc.vector.tensor_tensor(out=sss[:], in0=g16s[:],
                                            in1=ik_t[:, sl],
                                            op=mybir.AluOpType.mult)
                    nc.vector.tensor_tensor(out=sss[:], in0=sss[:],
                                            in1=egt[:, sl],
                                            op=mybir.AluOpType.add)
                    pn = ps_gn.tile([H, SUB], f32, tag="gn")
                    nc.tensor.matmul(pn[:], lhsT=ones16[:], rhs=sss[:],
                                     start=True, stop=True)
                    nrms = subp.tile([H, SUB], f32, tag="nrm")
                    nc.scalar.activation(nrms[:], pn[:],
                                         mybir.ActivationFunctionType.Identity,
                                         bias=eps16[:])
                    nc.vector.reciprocal(nrms[:], nrms[:])
                    ats = subp.tile([H, SUB], bf16, tag="attn")
                    nc.vector.tensor_tensor(out=ats[:], in0=sss[:], in1=nrms[:],
                                            op=mybir.AluOpType.mult)
                    for dc in range(DCH):
                        pr = ps_r.tile([P, SUB], f32, tag="pr")
                        nc.tensor.matmul(pr[:], lhsT=vwT_bf[:, dc * P:(dc + 1) * P],
                                         rhs=ats[:], start=True, stop=True)
                        rsb = subp.tile([P, SUB], bf16, tag="rsb")
                        if dc % 2 == 0:
                            nc.scalar.activation(
                                rsb[:], pr[:],
                                mybir.ActivationFunctionType.Identity,
                                bias=biasT[:, dc:dc + 1])
                        else:
                            nc.vector.tensor_scalar_add(
                                out=rsb[:], in0=pr[:],
                                scalar1=biasT[:, dc:dc + 1])
                        nc.sync.dma_start(
                            out=resT[dc * P:(dc + 1) * P, r0 + s0:r0 + s0 + SUB],
                            in_=rsb[:])

            # ---- phases 3+4: gathers with attention one pass behind ----
            Gts = {}
            for ps in range(NPASS):
                idxt, mskt, emapt = pre.pop(ps)
                if ps + 2 < NPASS:
                    pre[ps + 2] = load_gather_inputs(ps + 2)

                kv = kvp.tile([P, W], f32, tag="kv")
                nc.gpsimd.ap_gather(kv[:], table[:], idxt[:],
                                    channels=P, num_elems=NE, d=1, num_idxs=W)
                sbf = sbp.tile([P, W], bf16, tag="sbf")
                nc.vector.tensor_tensor_scan(
                    sbf[:], mskt[:], kv[:], 0.0,
                    op0=mybir.AluOpType.mult, op1=mybir.AluOpType.add)
                # local_scatter zero-fills and fully overwrites dst
                Gt = gtp.tile([P, CELLS], bf16, tag="g")
                nc.gpsimd.local_scatter(Gt[:], sbf[:], emapt[:],
                                        channels=P, num_elems=CELLS, num_idxs=W)
                Gts[ps] = Gt
                if ps >= 1:
                    # hoist the attention chunk's priority so the scheduler
                    # runs it inside the next gather's window instead of
                    # piling chunks after the last pass
                    with tc.high_priority(offset=400):
                        attn_chunk(ps - 1, Gts.pop(ps - 1))
            attn_chunk(NPASS - 1, Gts.pop(NPASS - 1))

    nc.compile()
    return nc


def prep_inputs(cfg: Cfg, x, adj_list, q_w, k_w, v_w, ego_scale, bias):
    """Host-side sharding/index prep.

    Returns (list of per-core input dicts, W). Sets cfg.W if unset.
    """
    import ml_dtypes
    N, D, H, K, NC = cfg.N, cfg.D, cfg.H, cfg.K, cfg.NC
    nr, C, NPASS, CELLS = cfg.nr, cfg.C, cfg.NPASS, cfg.CELLS

    src = np.asarray(adj_list[0], dtype=np.int64)
    dst = np.asarray(adj_list[1], dtype=np.int64)
    mk = np.asarray(adj_list[2]) != 0

    e_src, e_dst = src[mk], dst[mk]
    core = e_dst // nr
    lcell = e_dst % nr
    group = e_src // nr
    lsrc = (e_src % nr + 1).astype(np.int64)

    # balance cells across the 8 sub-passes to minimize the padded gather
    # width W = max over (core, group, pass) segment size
    cnt = np.zeros((NC, nr, 8), np.int32)
    np.add.at(cnt, (core, lcell, group), 1)
    perm = np.empty((NC, nr), np.int64)    # real cell -> column position
    for c in range(NC):
        tot = cnt[c].sum(1)
        order_c = np.argsort(-tot, kind="stable")
        loads = np.zeros((8, NPASS), np.int64)
        space = np.full(NPASS, CELLS)
        nxt = np.arange(NPASS) * CELLS
        for cell in order_c:
            v = cnt[c, cell][:, None]
            cand = (loads + v).max(0).astype(np.float64)
            cand[space == 0] = np.inf
            p = int(np.argmin(cand))
            loads[:, p] += cnt[c, cell]
            space[p] -= 1
            perm[c, cell] = nxt[p]
            nxt[p] += 1

    pos = perm[core, lcell]
    pss = pos // CELLS
    cell_in_ps = pos % CELLS

    key = ((core * 8 + group) * NPASS + pss) * 16384 + pos
    order = np.argsort(key, kind="stable")
    sc = core[order]
    sg = group[order]
    sp_ = pss[order]
    scell = cell_in_ps[order]
    ssrc = lsrc[order]

    seg = (sc * 8 + sg) * NPASS + sp_
    nseg = NC * 8 * NPASS
    seg_counts = np.bincount(seg, minlength=nseg)
    wmax = int(seg_counts.max())
    if cfg.W is None:
        cfg.W = max(16, -(-wmax // 16) * 16)
    W = cfg.W
    assert wmax <= W, f"W={W} too small for max segment {wmax}"
    seg_start = np.concatenate([[0], np.cumsum(seg_counts)])[:-1]
    pos = np.arange(len(order)) - seg_start[seg]

    same_prev = np.empty(len(order), dtype=bool)
    same_prev[0] = False
    same_prev[1:] = (seg[1:] == seg[:-1]) & (scell[1:] == scell[:-1])
    is_start = ~same_prev
    same_next = np.empty(len(order), dtype=bool)
    same_next[-1] = False
    same_next[:-1] = same_prev[1:]
    is_end = ~same_next

    idx8 = np.zeros((NC, NPASS, 8, W), np.int16)
    idx8[sc, sp_, sg, pos] = ssrc.astype(np.int16)
    msk8 = np.ones((NC, NPASS, 8, W), np.float32)
    msk8[sc[is_start], sp_[is_start], sg[is_start], pos[is_start]] = 0.0
    emap8 = np.full((NC, NPASS, 8, W), -1, np.int16)
    emap8[sc[is_end], sp_[is_end], sg[is_end], pos[is_end]] = \
        scell[is_end].astype(np.int16)

    # wrap idx streams: stream pos j -> (partition 16g + j%16, col j//16)
    idx_np = idx8.reshape(NC, NPASS, 8, W // 16, 16).transpose(0, 1, 2, 4, 3) \
        .reshape(NC, NPASS, P, W // 16)
    msk_np = np.repeat(msk8, 16, axis=2).astype(ml_dtypes.bfloat16)
    emap_np = np.repeat(emap8, 16, axis=2)

    ak = mk.reshape(N, K).sum(axis=1).astype(np.float32)
    inv_ak = (1.0 / (ak + 1e-6)).astype(np.float32)

    qkw_np = np.concatenate([np.asarray(q_w), np.asarray(k_w)],
                            axis=0).astype(np.float32)
    r16_np = (np.arange(P)[:, None] % 16 == np.arange(H)[None, :]) \
        .astype(ml_dtypes.bfloat16)

    in_maps = []
    for c in range(NC):
        xs = np.zeros((C, D), dtype=np.float32)
        xs[perm[c]] = np.asarray(x[c * nr:(c + 1) * nr], dtype=np.float32)
        xT = np.ascontiguousarray(xs.T).astype(ml_dtypes.bfloat16)

        ik = np.full((C,), 1e6, dtype=np.float32)
        ik[perm[c]] = inv_ak[c * nr:(c + 1) * nr]
        invk_np = np.broadcast_to(ik[None, :], (H, C)).copy()

        in_maps.append({
            "xT": xT,
            "qkw": qkw_np,
            "vw": np.asarray(v_w, dtype=np.float32),
            "ego": np.asarray(ego_scale, dtype=np.float32),
            "bias": np.asarray(bias, dtype=np.float32),
            "idx": idx_np[c],
            "msk": msk_np[c],
            "emap": emap_np[c],
            "invk": invk_np,
            "r16": r16_np,
        })
    return in_maps, W, perm


_CACHE = {}


def _get_compiled(cfg: Cfg):
    key = (cfg.N, cfg.D, cfg.H, cfg.K, cfg.NC, cfg.W)
    if key not in _CACHE:
        _CACHE[key] = build(cfg)
    return _CACHE[key]


def kernel(x, adj_list, q_w, k_w, v_w, ego_scale, bias, _trace=False):
    import concourse.bass_utils as bass_utils
    x = np.asarray(x)
    adj_list = np.asarray(adj_list)
    N, D = x.shape
    H = np.asarray(q_w).shape[0]
    K = adj_list.shape[1] // N
    cfg = Cfg(N=N, D=D, H=H, K=K, NC=8)

    dst = np.asarray(adj_list[1], dtype=np.int64)
    assert np.array_equal(dst, np.repeat(np.arange(N, dtype=np.int64), K)), \
        "kernel requires adj_list[1] grouped per target cell"

    in_maps, _, perm = prep_inputs(cfg, x, adj_list, q_w, k_w, v_w,
                                   ego_scale, bias)
    nc = _get_compiled(cfg)
    r = bass_utils.run_bass_kernel_spmd(nc, in_maps, core_ids=list(range(cfg.NC)),
                                        trace=_trace)
    out = np.concatenate(
        [r.results[c]["resT"].astype(np.float32).T[perm[c]]
         for c in range(cfg.NC)], axis=0)
    if _trace:
        return out, r
    return out



# revision 3
# speedup vs baseline: 1.1107x; 1.0832x over previous
"""Trainium2 Bass kernel for BilinearAttention GNN message passing.

Math (see reference):
  q = (x @ nonneg(q_w).T) / D ; k = (x @ nonneg(k_w).T) / D
  ego = q*q*nonneg(ego_scale)
  G[i,h] = sum_j mask[i,j] * k[src[i,j], h]          (dst grouped per cell)
  sum_local = q * G / (actual_k + 1e-6)
  s = ego + sum_local ; attn = s / (sum_h s + 1e-9)
  res = attn @ nonneg(v_w).T + nonneg(bias)

Distribution / algorithm (v2):
  Cells are sharded over 8 cores (12500 real -> 12544 padded each). Each
  core computes q and k^T for its cells, AllGathers k^T so that SBUF
  partition p holds column h=p%16 of source-core p//16's k table
  ("sections"). Valid edges are compacted on the host, bucketed per
  (source-section group, 1568-cell sub-pass), sorted by target cell, and
  gathered with gpsimd.ap_gather (all 8 Q7 cores in parallel, one 16-
  partition group per section). Per-cell sums come from a masked-reset
  prefix scan (state = m*state + kv, m=0 at run starts) followed by a
  gpsimd.local_scatter that drops each run-end value into the cell's
  column of G. A [128,16] block-indicator matmul reduces the 8 section
  groups, and the attention math + final matmul run on [16, cells]
  grids with the output produced transposed ([512, C] bf16).
"""

import sys

sys.path.insert(0, "/opt/trn_rl_repo")

import numpy as np

import concourse.bacc as bacc
import concourse.bass as bass
import concourse.mybir as mybir
import concourse.tile as tile
from concourse.masks import make_identity

P = 128


class Cfg:
    def __init__(self, N=100000, D=512, H=16, K=32, NC=8, W=None):
        assert N % NC == 0
        self.N, self.D, self.H, self.K, self.NC = N, D, H, K, NC
        self.nr = N // NC                  # real cells per core
        self.T = -(-self.nr // P)          # 128-row tiles per core
        self.C = self.T * P                # padded cells per core
        self.NE = self.C + 1               # table cols (zero col 0)
        self.NPASS = 16
        assert self.C % self.NPASS == 0
        self.CELLS = self.C // self.NPASS  # cells per sub-pass
        self.DCH = D // P
        self.W = W                         # gathered idx per (group, pass)
        self.CCW = 512                     # phase-1 cell chunk


def build(cfg: Cfg):
    f32, bf16, i16 = mybir.dt.float32, mybir.dt.bfloat16, mybir.dt.int16
    nc = bacc.Bacc("TRN2", target_bir_lowering=False, debug=False,
                   enable_asserts=False, num_devices=cfg.NC)
    D, H, C, NE, W = cfg.D, cfg.H, cfg.C, cfg.NE, cfg.W
    NPASS, CELLS, DCH = cfg.NPASS, cfg.CELLS, cfg.DCH

    xT_d = nc.dram_tensor("xT", [D, C], bf16, kind="ExternalInput")
    qkw_d = nc.dram_tensor("qkw", [2 * H, D], f32, kind="ExternalInput")
    vw_d = nc.dram_tensor("vw", [D, H], f32, kind="ExternalInput")
    ego_d = nc.dram_tensor("ego", [1, H], f32, kind="ExternalInput")
    bias_d = nc.dram_tensor("bias", [1, D], f32, kind="ExternalInput")
    idx_d = nc.dram_tensor("idx", [NPASS, P, W // 16], i16, kind="ExternalInput")
    msk_d = nc.dram_tensor("msk", [NPASS, P, W], bf16, kind="ExternalInput")
    emap_d = nc.dram_tensor("emap", [NPASS, P, W], i16, kind="ExternalInput")
    invk_d = nc.dram_tensor("invk", [H, C], f32, kind="ExternalInput")
    r16_d = nc.dram_tensor("r16", [P, H], bf16, kind="ExternalInput")

    qk_sh = nc.dram_tensor("qk_sh", [2 * H, C], f32, kind="Internal")
    kfullT = nc.dram_tensor("kfullT", [P, C], f32, kind="Internal",
                            addr_space="Shared")
    resT = nc.dram_tensor("resT", [D, C], bf16, kind="ExternalOutput")

    SUB = 392  # matmul free-dim sub-chunk inside a 1568-cell chunk

    with tile.TileContext(nc) as tc:
        with (
            tc.tile_pool(name="const", bufs=1) as cpool,
            tc.tile_pool(name="x_stream", bufs=3) as xpool,
            tc.tile_pool(name="stage", bufs=1) as stpool,
            tc.tile_pool(name="gidx", bufs=2) as gip,
            tc.tile_pool(name="gmsk", bufs=2) as gmp,
            tc.tile_pool(name="gemap", bufs=2) as gep,
            tc.tile_pool(name="kv", bufs=2) as kvp,
            tc.tile_pool(name="sbf", bufs=2) as sbp,
            tc.tile_pool(name="gtab", bufs=8) as gtp,
            tc.tile_pool(name="attn", bufs=1) as apool,
            tc.tile_pool(name="subs", bufs=4) as subp,
            tc.tile_pool(name="ps_qk", bufs=2, space="PSUM") as ps_qk,
            tc.tile_pool(name="ps_t", bufs=2, space="PSUM") as ps_t,
            tc.tile_pool(name="ps_gn", bufs=2, space="PSUM") as ps_gn,
            tc.tile_pool(name="ps_r", bufs=2, space="PSUM") as ps_r,
        ):
            ident = cpool.tile([P, P], f32)
            make_identity(nc, ident[:])

            def nonneg(dst, src, tmp):
                # elu(w)+1 = exp(min(w,0)) + max(w,0)
                nc.vector.tensor_scalar_min(tmp[:], src[:], 0.0)
                nc.scalar.activation(tmp[:], tmp[:],
                                     mybir.ActivationFunctionType.Exp)
                nc.vector.tensor_scalar_max(dst[:], src[:], 0.0)
                nc.vector.tensor_add(dst[:], dst[:], tmp[:])

            # ---- weight prep (temporaries live in the stage pool) ----
            qkw_sb = stpool.tile([2 * H, D], f32, tag="w_src")
            nc.sync.dma_start(out=qkw_sb[:], in_=qkw_d[:, :])
            qkw_nn = stpool.tile([2 * H, D], f32, tag="w_nn")
            tmp_a = stpool.tile([2 * H, D], f32, tag="w_tmp")
            nonneg(qkw_nn, qkw_sb, tmp_a)
            nc.vector.tensor_scalar_mul(qkw_nn[:], qkw_nn[:], 1.0 / D)
            qkwT_bf = cpool.tile([P, DCH, 2 * H], bf16)
            for dc in range(DCH):
                pt = ps_t.tile([P, P], f32, tag="tp")
                nc.tensor.transpose(out=pt[:, 0:2 * H],
                                    in_=qkw_nn[:, dc * P:(dc + 1) * P],
                                    identity=ident[:2 * H, :2 * H])
                nc.vector.tensor_copy(qkwT_bf[:, dc, :], pt[:, 0:2 * H])

            vwT = stpool.tile([H, D], f32, tag="w_src")
            for dc in range(DCH):
                vc = stpool.tile([P, H], f32, tag="vld")
                nc.sync.dma_start(out=vc[:], in_=vw_d[dc * P:(dc + 1) * P, :])
                pt2 = ps_t.tile([P, P], f32, tag="tp")
                nc.tensor.transpose(out=pt2[0:H, :], in_=vc[:], identity=ident[:])
                nc.vector.tensor_copy(vwT[:, dc * P:(dc + 1) * P], pt2[0:H, :])
            vwT_nn = stpool.tile([H, D], f32, tag="w_nn2")
            tmp_v = stpool.tile([H, D], f32, tag="w_tmp")
            nonneg(vwT_nn, vwT, tmp_v)
            vwT_bf = cpool.tile([H, D], bf16)
            nc.vector.tensor_copy(vwT_bf[:], vwT_nn[:])

            bias_sb = stpool.tile([1, D], f32, tag="w_src")
            nc.sync.dma_start(out=bias_sb[:], in_=bias_d[:, :])
            bias_nn = stpool.tile([1, D], f32, tag="w_nn")
            tmp_b = stpool.tile([1, D], f32, tag="w_tmp")
            nonneg(bias_nn, bias_sb, tmp_b)
            # per-partition bias column per d-chunk (consumed as activation
            # bias on the output copy; replaces the 1-contraction matmuls)
            biasT = cpool.tile([P, DCH], f32)
            for dc in range(DCH):
                ptb = ps_t.tile([P, P], f32, tag="tp")
                nc.tensor.transpose(out=ptb[:, 0:1],
                                    in_=bias_nn[:, dc * P:(dc + 1) * P],
                                    identity=ident[:1, :1])
                nc.vector.tensor_copy(biasT[:, dc:dc + 1], ptb[:, 0:1])

            ego_sb = stpool.tile([1, H], f32, tag="w_src")
            nc.sync.dma_start(out=ego_sb[:], in_=ego_d[:, :])
            ego_nn = stpool.tile([1, H], f32, tag="w_nn")
            tmp_g = stpool.tile([1, H], f32, tag="w_tmp")
            nonneg(ego_nn, ego_sb, tmp_g)
            pe = ps_t.tile([P, P], f32, tag="tp")
            nc.tensor.transpose(out=pe[0:H, 0:1], in_=ego_nn[:],
                                identity=ident[:1, :1])
            ego_rt = cpool.tile([H, 1], f32)
            nc.scalar.activation(ego_rt[:], pe[0:H, 0:1],
                                 mybir.ActivationFunctionType.Sqrt)

            ones16 = cpool.tile([H, H], f32)
            nc.vector.memset(ones16[:], 1.0)
            eps16 = cpool.tile([H, 1], f32)
            nc.vector.memset(eps16[:], 1e-9)
            r16 = cpool.tile([P, H], bf16)
            nc.sync.dma_start(out=r16[:], in_=r16_d[:, :])

            # ---- phase 1a: k embeddings only (critical path to AllGather) ----
            def qk_half(c0, cw, lo, hi, dst_rows):
                xt = xpool.tile([P, DCH, cfg.CCW], bf16, tag="xt")
                nc.sync.dma_start(
                    out=xt[:, :, :cw],
                    in_=xT_d[:, c0:c0 + cw].rearrange("(dc p) c -> p dc c", p=P))
                pqk = ps_qk.tile([H, cfg.CCW], f32, tag="pqk")
                for dc in range(DCH):
                    nc.tensor.matmul(pqk[:, :cw], lhsT=qkwT_bf[:, dc, lo:hi],
                                     rhs=xt[:, dc, :cw],
                                     start=(dc == 0), stop=(dc == DCH - 1))
                stg = stpool.tile([H, cfg.CCW], f32, tag="stg")
                nc.vector.tensor_copy(stg[:, :cw], pqk[:, :cw])
                nc.sync.dma_start(out=qk_sh[dst_rows, c0:c0 + cw],
                                  in_=stg[:, :cw])

            for c0 in range(0, C, cfg.CCW):
                cw = min(cfg.CCW, C - c0)
                qk_half(c0, cw, H, 2 * H, slice(H, 2 * H))

            # ---- AllGather k^T ----
            nc.gpsimd.collective_compute(
                "AllGather", mybir.AluOpType.bypass,
                replica_groups=[list(range(cfg.NC))],
                ins=[qk_sh[H:2 * H, :].opt()],
                outs=[kfullT[:, :].opt()],
            )

            table = cpool.tile([P, NE], f32)
            nc.vector.memset(table[:, 0:1], 0.0)
            nc.sync.dma_start(out=table[:, 1:NE], in_=kfullT[:, :])

            # gather-input prefetch (2 passes ahead)
            def load_gather_inputs(ps):
                idxt = gip.tile([P, W // 16], i16, tag="idx")
                nc.sync.dma_start(out=idxt[:], in_=idx_d[ps, :, :])
                mskt = gmp.tile([P, W], bf16, tag="msk")
                nc.sync.dma_start(out=mskt[:], in_=msk_d[ps, :, :])
                emapt = gep.tile([P, W], i16, tag="emap")
                nc.sync.dma_start(out=emapt[:], in_=emap_d[ps, :, :])
                return idxt, mskt, emapt

            pre = {0: load_gather_inputs(0), 1: load_gather_inputs(1)}

            # ---- phase 1b: q embeddings (overlaps the gather passes) ----
            for c0 in range(0, C, cfg.CCW):
                cw = min(cfg.CCW, C - c0)
                qk_half(c0, cw, 0, H, slice(0, H))

            # ---- attention + output for one 1568-cell chunk ----
            def attn_chunk(ch, Gt):
                r0 = ch * CELLS
                q_t = apool.tile([H, CELLS], f32, tag="q")
                nc.sync.dma_start(out=q_t[:], in_=qk_sh[0:H, r0:r0 + CELLS])
                ik_t = apool.tile([H, CELLS], f32, tag="ik")
                nc.sync.dma_start(out=ik_t[:], in_=invk_d[:, r0:r0 + CELLS])

                nc.vector.tensor_tensor(out=ik_t[:], in0=q_t[:], in1=ik_t[:],
                                        op=mybir.AluOpType.mult)
                egt = apool.tile([H, CELLS], f32, tag="egt")
                nc.scalar.activation(egt[:], q_t[:],
                                     mybir.ActivationFunctionType.Square,
                                     scale=ego_rt[:])

                for s0 in range(0, CELLS, SUB):
                    sl = slice(s0, s0 + SUB)
                    pg = ps_gn.tile([H, SUB], f32, tag="gn")
                    nc.tensor.matmul(pg[:], lhsT=r16[:], rhs=Gt[:, sl],
                                     start=True, stop=True)
                    # DVE reads PSUM ~6x slower than SBUF; stage via ACT
                    g16s = subp.tile([H, SUB], f32, tag="g16")
                    nc.scalar.activation(g16s[:], pg[:],
                                         mybir.ActivationFunctionType.Copy)
                    sss = subp.tile([H, SUB], f32, tag="ss")
                    nc.vector.tensor_tensor(out=sss[:], in0=g16s[:],
                                            in1=ik_t[:, sl],
                                            op=mybir.AluOpType.mult)
                    nc.vector.tensor_tensor(out=sss[:], in0=sss[:],
                                            in1=egt[:, sl],
                                            op=mybir.AluOpType.add)
                    pn = ps_gn.tile([H, SUB], f32, tag="gn")
                    nc.tensor.matmul(pn[:], lhsT=ones16[:], rhs=sss[:],
                                     start=True, stop=True)
                    nrms = subp.tile([H, SUB], f32, tag="nrm")
                    nc.scalar.activation(nrms[:], pn[:],
                                         mybir.ActivationFunctionType.Identity,
                                         bias=eps16[:])
                    nc.vector.reciprocal(nrms[:], nrms[:])
                    ats = subp.tile([H, SUB], bf16, tag="attn")
                    nc.vector.tensor_tensor(out=ats[:], in0=sss[:], in1=nrms[:],
                                            op=mybir.AluOpType.mult)
                    for dc in range(DCH):
                        pr = ps_r.tile([P, SUB], f32, tag="pr")
                        nc.tensor.matmul(pr[:], lhsT=vwT_bf[:, dc * P:(dc + 1) * P],
                                         rhs=ats[:], start=True, stop=True)
                        rsb = subp.tile([P, SUB], bf16, tag="rsb")
                        if dc % 2 == 0:
                            nc.scalar.activation(
                                rsb[:], pr[:],
                                mybir.ActivationFunctionType.Identity,
                                bias=biasT[:, dc:dc + 1])
                        else:
                            nc.vector.tensor_scalar_add(
                                out=rsb[:], in0=pr[:],
                                scalar1=biasT[:, dc:dc + 1])
                        nc.sync.dma_start(
                            out=resT[dc * P:(dc + 1) * P, r0 + s0:r0 + s0 + SUB],
                            in_=rsb[:])

            # ---- phases 3+4: gathers with attention one pass behind ----
            Gts = {}
            for ps in range(NPASS):
                idxt, mskt, emapt = pre.pop(ps)
                if ps + 2 < NPASS:
                    pre[ps + 2] = load_gather_inputs(ps + 2)

                kv = kvp.tile([P, W], f32, tag="kv")
                nc.gpsimd.ap_gather(kv[:], table[:], idxt[:],
                                    channels=P, num_elems=NE, d=1, num_idxs=W)
                sbf = sbp.tile([P, W], bf16, tag="sbf")
                nc.vector.tensor_tensor_scan(
                    sbf[:], mskt[:], kv[:], 0.0,
                    op0=mybir.AluOpType.mult, op1=mybir.AluOpType.add)
                # local_scatter zero-fills and fully overwrites dst
                Gt = gtp.tile([P, CELLS], bf16, tag="g")
                nc.gpsimd.local_scatter(Gt[:], sbf[:], emapt[:],
                                        channels=P, num_elems=CELLS, num_idxs=W)
                Gts[ps] = Gt
                if ps >= 1:
                    # hoist the attention chunk's priority so the scheduler
                    # runs it inside the next gather's window instead of
                    # piling chunks after the last pass
                    with tc.high_priority(offset=400):
                        attn_chunk(ps - 1, Gts.pop(ps - 1))
            attn_chunk(NPASS - 1, Gts.pop(NPASS - 1))

    nc.compile()
    return nc


def prep_inputs(cfg: Cfg, x, adj_list, q_w, k_w, v_w, ego_scale, bias):
    """Host-side sharding/index prep.

    Returns (list of per-core input dicts, W). Sets cfg.W if unset.
    """
    import ml_dtypes
    N, D, H, K, NC = cfg.N, cfg.D, cfg.H, cfg.K, cfg.NC
    nr, C, NPASS, CELLS = cfg.nr, cfg.C, cfg.NPASS, cfg.CELLS

    src = np.asarray(adj_list[0], dtype=np.int64)
    dst = np.asarray(adj_list[1], dtype=np.int64)
    mk = np.asarray(adj_list[2]) != 0

    e_src, e_dst = src[mk], dst[mk]
    core = e_dst // nr
    lcell = e_dst % nr
    group = e_src // nr
    lsrc = (e_src % nr + 1).astype(np.int64)

    # balance cells across the 8 sub-passes to minimize the padded gather
    # width W = max over (core, group, pass) segment size
    cnt = np.zeros((NC, nr, 8), np.int32)
    np.add.at(cnt, (core, lcell, group), 1)
    perm = np.empty((NC, nr), np.int64)    # real cell -> column position
    for c in range(NC):
        tot = cnt[c].sum(1)
        order_c = np.argsort(-tot, kind="stable")
        loads = np.zeros((8, NPASS), np.int64)
        space = np.full(NPASS, CELLS)
        nxt = np.arange(NPASS) * CELLS
        for cell in order_c:
            v = cnt[c, cell][:, None]
            cand = (loads + v).max(0).astype(np.float64)
            cand[space == 0] = np.inf
            p = int(np.argmin(cand))
            loads[:, p] += cnt[c, cell]
            space[p] -= 1
            perm[c, cell] = nxt[p]
            nxt[p] += 1

    pos = perm[core, lcell]
    pss = pos // CELLS
    cell_in_ps = pos % CELLS

    key = ((core * 8 + group) * NPASS + pss) * 16384 + pos
    order = np.argsort(key, kind="stable")
    sc = core[order]
    sg = group[order]
    sp_ = pss[order]
    scell = cell_in_ps[order]
    ssrc = lsrc[order]

    seg = (sc * 8 + sg) * NPASS + sp_
    nseg = NC * 8 * NPASS
    seg_counts = np.bincount(seg, minlength=nseg)
    wmax = int(seg_counts.max())
    if cfg.W is None:
        cfg.W = max(16, -(-wmax // 16) * 16)
    W = cfg.W
    assert wmax <= W, f"W={W} too small for max segment {wmax}"
    seg_start = np.concatenate([[0], np.cumsum(seg_counts)])[:-1]
    pos = np.arange(len(order)) - seg_start[seg]

    same_prev = np.empty(len(order), dtype=bool)
    same_prev[0] = False
    same_prev[1:] = (seg[1:] == seg[:-1]) & (scell[1:] == scell[:-1])
    is_start = ~same_prev
    same_next = np.empty(len(order), dtype=bool)
    same_next[-1] = False
    same_next[:-1] = same_prev[1:]
    is_end = ~same_next

    idx8 = np.zeros((NC, NPASS, 8, W), np.int16)
    idx8[sc, sp_, sg, pos] = ssrc.astype(np.int16)
    msk8 = np.ones((NC, NPASS, 8, W), np.float32)
    msk8[sc[is_start], sp_[is_start], sg[is_start], pos[is_start]] = 0.0
    emap8 = np.full((NC, NPASS, 8, W), -1, np.int16)
    emap8[sc[is_end], sp_[is_end], sg[is_end], pos[is_end]] = \
        scell[is_end].astype(np.int16)

    # wrap idx streams: stream pos j -> (partition 16g + j%16, col j//16)
    idx_np = idx8.reshape(NC, NPASS, 8, W // 16, 16).transpose(0, 1, 2, 4, 3) \
        .reshape(NC, NPASS, P, W // 16)
    msk_np = np.repeat(msk8, 16, axis=2).astype(ml_dtypes.bfloat16)
    emap_np = np.repeat(emap8, 16, axis=2)

    ak = mk.reshape(N, K).sum(axis=1).astype(np.float32)
    inv_ak = (1.0 / (ak + 1e-6)).astype(np.float32)

    qkw_np = np.concatenate([np.asarray(q_w), np.asarray(k_w)],
                            axis=0).astype(np.float32)
    r16_np = (np.arange(P)[:, None] % 16 == np.arange(H)[None, :]) \
        .astype(ml_dtypes.bfloat16)

    in_maps = []
    for c in range(NC):
        xs = np.zeros((C, D), dtype=np.float32)
        xs[perm[c]] = np.asarray(x[c * nr:(c + 1) * nr], dtype=np.float32)
        xT = np.ascontiguousarray(xs.T).astype(ml_dtypes.bfloat16)

        ik = np.full((C,), 1e6, dtype=np.float32)
        ik[perm[c]] = inv_ak[c * nr:(c + 1) * nr]
        invk_np = np.broadcast_to(ik[None, :], (H, C)).copy()

        in_maps.append({
            "xT": xT,
            "qkw": qkw_np,
            "vw": np.asarray(v_w, dtype=np.float32),
            "ego": np.asarray(ego_scale, dtype=np.float32),
            "bias": np.asarray(bias, dtype=np.float32),
            "idx": idx_np[c],
            "msk": msk_np[c],
            "emap": emap_np[c],
            "invk": invk_np,
            "r16": r16_np,
        })
    return in_maps, W, perm


_CACHE = {}


def _get_compiled(cfg: Cfg):
    key = (cfg.N, cfg.D, cfg.H, cfg.K, cfg.NC, cfg.W)
    if key not in _CACHE:
        _CACHE[key] = build(cfg)
    return _CACHE[key]


def kernel(x, adj_list, q_w, k_w, v_w, ego_scale, bias, _trace=False):
    import concourse.bass_utils as bass_utils
    x = np.asarray(x)
    adj_list = np.asarray(adj_list)
    N, D = x.shape
    H = np.asarray(q_w).shape[0]
    K = adj_list.shape[1] // N
    cfg = Cfg(N=N, D=D, H=H, K=K, NC=8)

    dst = np.asarray(adj_list[1], dtype=np.int64)
    assert np.array_equal(dst, np.repeat(np.arange(N, dtype=np.int64), K)), \
        "kernel requires adj_list[1] grouped per target cell"

    in_maps, _, perm = prep_inputs(cfg, x, adj_list, q_w, k_w, v_w,
                                   ego_scale, bias)
    nc = _get_compiled(cfg)
    r = bass_utils.run_bass_kernel_spmd(nc, in_maps, core_ids=list(range(cfg.NC)),
                                        trace=_trace)
    out = np.concatenate(
        [r.results[c]["resT"].astype(np.float32).T[perm[c]]
         for c in range(cfg.NC)], axis=0)
    if _trace:
        return out, r
    return out



# revision 4
# speedup vs baseline: 1.1394x; 1.0259x over previous
"""Trainium2 Bass kernel for BilinearAttention GNN message passing.

Math (see reference):
  q = (x @ nonneg(q_w).T) / D ; k = (x @ nonneg(k_w).T) / D
  ego = q*q*nonneg(ego_scale)
  G[i,h] = sum_j mask[i,j] * k[src[i,j], h]          (dst grouped per cell)
  sum_local = q * G / (actual_k + 1e-6)
  s = ego + sum_local ; attn = s / (sum_h s + 1e-9)
  res = attn @ nonneg(v_w).T + nonneg(bias)

Distribution / algorithm (v2):
  Cells are sharded over 8 cores (12500 real -> 12544 padded each). Each
  core computes q and k^T for its cells, AllGathers k^T so that SBUF
  partition p holds column h=p%16 of source-core p//16's k table
  ("sections"). Valid edges are compacted on the host, bucketed per
  (source-section group, 1568-cell sub-pass), sorted by target cell, and
  gathered with gpsimd.ap_gather (all 8 Q7 cores in parallel, one 16-
  partition group per section). Per-cell sums come from a masked-reset
  prefix scan (state = m*state + kv, m=0 at run starts) followed by a
  gpsimd.local_scatter that drops each run-end value into the cell's
  column of G. A [128,16] block-indicator matmul reduces the 8 section
  groups, and the attention math + final matmul run on [16, cells]
  grids with the output produced transposed ([512, C] bf16).
"""

import sys

sys.path.insert(0, "/opt/trn_rl_repo")

import numpy as np

import concourse.bacc as bacc
import concourse.bass as bass
import concourse.mybir as mybir
import concourse.tile as tile
from concourse.masks import make_identity

P = 128


class Cfg:
    def __init__(self, N=100000, D=512, H=16, K=32, NC=8, W=None):
        assert N % NC == 0
        self.N, self.D, self.H, self.K, self.NC = N, D, H, K, NC
        self.nr = N // NC                  # real cells per core
        self.T = -(-self.nr // P)          # 128-row tiles per core
        self.C = self.T * P                # padded cells per core
        self.NE = self.C + 1               # table cols (zero col 0)
        self.NPASS = 16
        assert self.C % self.NPASS == 0
        self.CELLS = self.C // self.NPASS  # cells per sub-pass
        self.DCH = D // P
        self.W = W                         # gathered idx per (group, pass)
        self.CCW = 512                     # phase-1 cell chunk


def build(cfg: Cfg):
    f32, bf16, i16 = mybir.dt.float32, mybir.dt.bfloat16, mybir.dt.int16
    nc = bacc.Bacc("TRN2", target_bir_lowering=False, debug=False,
                   enable_asserts=False, num_devices=cfg.NC)
    D, H, C, NE, W = cfg.D, cfg.H, cfg.C, cfg.NE, cfg.W
    NPASS, CELLS, DCH = cfg.NPASS, cfg.CELLS, cfg.DCH

    xT_d = nc.dram_tensor("xT", [D, C], bf16, kind="ExternalInput")
    qkw_d = nc.dram_tensor("qkw", [2 * H, D], f32, kind="ExternalInput")
    vw_d = nc.dram_tensor("vw", [D, H], f32, kind="ExternalInput")
    ego_d = nc.dram_tensor("ego", [1, H], f32, kind="ExternalInput")
    bias_d = nc.dram_tensor("bias", [1, D], f32, kind="ExternalInput")
    idx_d = nc.dram_tensor("idx", [NPASS, P, W // 16], i16, kind="ExternalInput")
    msk_d = nc.dram_tensor("msk", [NPASS, P, W], bf16, kind="ExternalInput")
    emap_d = nc.dram_tensor("emap", [NPASS, P, W], i16, kind="ExternalInput")
    invk_d = nc.dram_tensor("invk", [H, C], f32, kind="ExternalInput")
    r16_d = nc.dram_tensor("r16", [P, H], bf16, kind="ExternalInput")

    qk_sh = nc.dram_tensor("qk_sh", [2 * H, C], f32, kind="Internal")
    CH = (C // (2 * 512)) * 512  # first AllGather half (CCW-aligned)
    ksh_a = nc.dram_tensor("ksh_a", [H, CH], f32, kind="Internal")
    ksh_b = nc.dram_tensor("ksh_b", [H, C - CH], f32, kind="Internal")
    kf_a = nc.dram_tensor("kf_a", [P, CH], f32, kind="Internal",
                          addr_space="Shared")
    kf_b = nc.dram_tensor("kf_b", [P, C - CH], f32, kind="Internal",
                          addr_space="Shared")
    resT = nc.dram_tensor("resT", [D, C], bf16, kind="ExternalOutput")

    SUB = 392  # matmul free-dim sub-chunk inside a 1568-cell chunk

    with tile.TileContext(nc) as tc:
        with (
            tc.tile_pool(name="const", bufs=1) as cpool,
            tc.tile_pool(name="x_stream", bufs=3) as xpool,
            tc.tile_pool(name="stage", bufs=1) as stpool,
            tc.tile_pool(name="gidx", bufs=2) as gip,
            tc.tile_pool(name="gmsk", bufs=2) as gmp,
            tc.tile_pool(name="gemap", bufs=2) as gep,
            tc.tile_pool(name="kv", bufs=2) as kvp,
            tc.tile_pool(name="sbf", bufs=2) as sbp,
            tc.tile_pool(name="gtab", bufs=8) as gtp,
            tc.tile_pool(name="attn", bufs=1) as apool,
            tc.tile_pool(name="subs", bufs=4) as subp,
            tc.tile_pool(name="ps_qk", bufs=2, space="PSUM") as ps_qk,
            tc.tile_pool(name="ps_t", bufs=2, space="PSUM") as ps_t,
            tc.tile_pool(name="ps_gn", bufs=2, space="PSUM") as ps_gn,
            tc.tile_pool(name="ps_r", bufs=2, space="PSUM") as ps_r,
        ):
            ident = cpool.tile([P, P], f32)
            make_identity(nc, ident[:])

            def nonneg(dst, src, tmp):
                # elu(w)+1 = exp(min(w,0)) + max(w,0)
                nc.vector.tensor_scalar_min(tmp[:], src[:], 0.0)
                nc.scalar.activation(tmp[:], tmp[:],
                                     mybir.ActivationFunctionType.Exp)
                nc.vector.tensor_scalar_max(dst[:], src[:], 0.0)
                nc.vector.tensor_add(dst[:], dst[:], tmp[:])

            # ---- weight prep (temporaries live in the stage pool) ----
            qkw_sb = stpool.tile([2 * H, D], f32, tag="w_src")
            nc.sync.dma_start(out=qkw_sb[:], in_=qkw_d[:, :])
            qkw_nn = stpool.tile([2 * H, D], f32, tag="w_nn")
            tmp_a = stpool.tile([2 * H, D], f32, tag="w_tmp")
            nonneg(qkw_nn, qkw_sb, tmp_a)
            nc.vector.tensor_scalar_mul(qkw_nn[:], qkw_nn[:], 1.0 / D)
            qkwT_bf = cpool.tile([P, DCH, 2 * H], bf16)
            for dc in range(DCH):
                pt = ps_t.tile([P, P], f32, tag="tp")
                nc.tensor.transpose(out=pt[:, 0:2 * H],
                                    in_=qkw_nn[:, dc * P:(dc + 1) * P],
                                    identity=ident[:2 * H, :2 * H])
                nc.vector.tensor_copy(qkwT_bf[:, dc, :], pt[:, 0:2 * H])

            vwT = stpool.tile([H, D], f32, tag="w_src")
            for dc in range(DCH):
                vc = stpool.tile([P, H], f32, tag="vld")
                nc.sync.dma_start(out=vc[:], in_=vw_d[dc * P:(dc + 1) * P, :])
                pt2 = ps_t.tile([P, P], f32, tag="tp")
                nc.tensor.transpose(out=pt2[0:H, :], in_=vc[:], identity=ident[:])
                nc.vector.tensor_copy(vwT[:, dc * P:(dc + 1) * P], pt2[0:H, :])
            vwT_nn = stpool.tile([H, D], f32, tag="w_nn2")
            tmp_v = stpool.tile([H, D], f32, tag="w_tmp")
            nonneg(vwT_nn, vwT, tmp_v)
            vwT_bf = cpool.tile([H, D], bf16)
            nc.vector.tensor_copy(vwT_bf[:], vwT_nn[:])

            bias_sb = stpool.tile([1, D], f32, tag="w_src")
            nc.sync.dma_start(out=bias_sb[:], in_=bias_d[:, :])
            bias_nn = stpool.tile([1, D], f32, tag="w_nn")
            tmp_b = stpool.tile([1, D], f32, tag="w_tmp")
            nonneg(bias_nn, bias_sb, tmp_b)
            # per-partition bias column per d-chunk (consumed as activation
            # bias on the output copy; replaces the 1-contraction matmuls)
            biasT = cpool.tile([P, DCH], f32)
            for dc in range(DCH):
                ptb = ps_t.tile([P, P], f32, tag="tp")
                nc.tensor.transpose(out=ptb[:, 0:1],
                                    in_=bias_nn[:, dc * P:(dc + 1) * P],
                                    identity=ident[:1, :1])
                nc.vector.tensor_copy(biasT[:, dc:dc + 1], ptb[:, 0:1])

            ego_sb = stpool.tile([1, H], f32, tag="w_src")
            nc.sync.dma_start(out=ego_sb[:], in_=ego_d[:, :])
            ego_nn = stpool.tile([1, H], f32, tag="w_nn")
            tmp_g = stpool.tile([1, H], f32, tag="w_tmp")
            nonneg(ego_nn, ego_sb, tmp_g)
            pe = ps_t.tile([P, P], f32, tag="tp")
            nc.tensor.transpose(out=pe[0:H, 0:1], in_=ego_nn[:],
                                identity=ident[:1, :1])
            ego_rt = cpool.tile([H, 1], f32)
            nc.scalar.activation(ego_rt[:], pe[0:H, 0:1],
                                 mybir.ActivationFunctionType.Sqrt)

            ones16 = cpool.tile([H, H], f32)
            nc.vector.memset(ones16[:], 1.0)
            eps16 = cpool.tile([H, 1], f32)
            nc.vector.memset(eps16[:], 1e-9)
            r16 = cpool.tile([P, H], bf16)
            nc.sync.dma_start(out=r16[:], in_=r16_d[:, :])

            # ---- phase 1a: k embeddings only (critical path to AllGather) ----
            def qk_half(c0, cw, lo, hi, dst_rows):
                xt = xpool.tile([P, DCH, cfg.CCW], bf16, tag="xt")
                nc.sync.dma_start(
                    out=xt[:, :, :cw],
                    in_=xT_d[:, c0:c0 + cw].rearrange("(dc p) c -> p dc c", p=P))
                pqk = ps_qk.tile([H, cfg.CCW], f32, tag="pqk")
                for dc in range(DCH):
                    nc.tensor.matmul(pqk[:, :cw], lhsT=qkwT_bf[:, dc, lo:hi],
                                     rhs=xt[:, dc, :cw],
                                     start=(dc == 0), stop=(dc == DCH - 1))
                stg = stpool.tile([H, cfg.CCW], f32, tag="stg")
                nc.vector.tensor_copy(stg[:, :cw], pqk[:, :cw])
                if dst_rows is None:   # k half -> split shard tensors
                    if c0 < CH:
                        nc.sync.dma_start(out=ksh_a[:, c0:c0 + cw],
                                          in_=stg[:, :cw])
                    else:
                        nc.sync.dma_start(
                            out=ksh_b[:, c0 - CH:c0 - CH + cw],
                            in_=stg[:, :cw])
                else:
                    nc.sync.dma_start(out=qk_sh[dst_rows, c0:c0 + cw],
                                      in_=stg[:, :cw])

            # k first half, then AllGather it while the second half computes
            for c0 in range(0, CH, cfg.CCW):
                qk_half(c0, cfg.CCW, H, 2 * H, None)
            nc.gpsimd.collective_compute(
                "AllGather", mybir.AluOpType.bypass,
                replica_groups=[list(range(cfg.NC))],
                ins=[ksh_a[:, :].opt()],
                outs=[kf_a[:, :].opt()],
            )
            for c0 in range(CH, C, cfg.CCW):
                cw = min(cfg.CCW, C - c0)
                qk_half(c0, cw, H, 2 * H, None)
            nc.gpsimd.collective_compute(
                "AllGather", mybir.AluOpType.bypass,
                replica_groups=[list(range(cfg.NC))],
                ins=[ksh_b[:, :].opt()],
                outs=[kf_b[:, :].opt()],
            )

            table = cpool.tile([P, NE], f32)
            nc.vector.memset(table[:, 0:1], 0.0)
            nc.sync.dma_start(out=table[:, 1:1 + CH], in_=kf_a[:, :])
            nc.sync.dma_start(out=table[:, 1 + CH:NE], in_=kf_b[:, :])

            # gather-input prefetch (2 passes ahead)
            def load_gather_inputs(ps):
                idxt = gip.tile([P, W // 16], i16, tag="idx")
                nc.sync.dma_start(out=idxt[:], in_=idx_d[ps, :, :])
                mskt = gmp.tile([P, W], bf16, tag="msk")
                nc.sync.dma_start(out=mskt[:], in_=msk_d[ps, :, :])
                emapt = gep.tile([P, W], i16, tag="emap")
                nc.sync.dma_start(out=emapt[:], in_=emap_d[ps, :, :])
                return idxt, mskt, emapt

            pre = {0: load_gather_inputs(0), 1: load_gather_inputs(1)}

            # ---- phase 1b: q embeddings (overlaps the gather passes) ----
            for c0 in range(0, C, cfg.CCW):
                cw = min(cfg.CCW, C - c0)
                qk_half(c0, cw, 0, H, slice(0, H))

            # ---- attention + output for one 1568-cell chunk ----
            def attn_chunk(ch, Gt):
                r0 = ch * CELLS
                q_t = apool.tile([H, CELLS], f32, tag="q")
                nc.sync.dma_start(out=q_t[:], in_=qk_sh[0:H, r0:r0 + CELLS])
                ik_t = apool.tile([H, CELLS], f32, tag="ik")
                nc.sync.dma_start(out=ik_t[:], in_=invk_d[:, r0:r0 + CELLS])

                nc.vector.tensor_tensor(out=ik_t[:], in0=q_t[:], in1=ik_t[:],
                                        op=mybir.AluOpType.mult)
                egt = apool.tile([H, CELLS], f32, tag="egt")
                nc.scalar.activation(egt[:], q_t[:],
                                     mybir.ActivationFunctionType.Square,
                                     scale=ego_rt[:])

                for s0 in range(0, CELLS, SUB):
                    sl = slice(s0, s0 + SUB)
                    pg = ps_gn.tile([H, SUB], f32, tag="gn")
                    nc.tensor.matmul(pg[:], lhsT=r16[:], rhs=Gt[:, sl],
                                     start=True, stop=True)
                    # DVE reads PSUM ~6x slower than SBUF; stage via ACT
                    g16s = subp.tile([H, SUB], f32, tag="g16")
                    nc.scalar.activation(g16s[:], pg[:],
                                         mybir.ActivationFunctionType.Copy)
                    sss = subp.tile([H, SUB], f32, tag="ss")
                    nc.vector.tensor_tensor(out=sss[:], in0=g16s[:],
                                            in1=ik_t[:, sl],
                                            op=mybir.AluOpType.mult)
                    nc.vector.tensor_tensor(out=sss[:], in0=sss[:],
                                            in1=egt[:, sl],
                                            op=mybir.AluOpType.add)
                    pn = ps_gn.tile([H, SUB], f32, tag="gn")
                    nc.tensor.matmul(pn[:], lhsT=ones16[:], rhs=sss[:],
                                     start=True, stop=True)
                    nrms = subp.tile([H, SUB], f32, tag="nrm")
                    nc.scalar.activation(nrms[:], pn[:],
                                         mybir.ActivationFunctionType.Identity,
                                         bias=eps16[:])
                    nc.vector.reciprocal(nrms[:], nrms[:])
                    ats = subp.tile([H, SUB], bf16, tag="attn")
                    nc.vector.tensor_tensor(out=ats[:], in0=sss[:], in1=nrms[:],
                                            op=mybir.AluOpType.mult)
                    for dc in range(DCH):
                        pr = ps_r.tile([P, SUB], f32, tag="pr")
                        nc.tensor.matmul(pr[:], lhsT=vwT_bf[:, dc * P:(dc + 1) * P],
                                         rhs=ats[:], start=True, stop=True)
                        rsb = subp.tile([P, SUB], bf16, tag="rsb")
                        if dc % 2 == 0:
                            nc.scalar.activation(
                                rsb[:], pr[:],
                                mybir.ActivationFunctionType.Identity,
                                bias=biasT[:, dc:dc + 1])
                        else:
                            nc.vector.tensor_scalar_add(
                                out=rsb[:], in0=pr[:],
                                scalar1=biasT[:, dc:dc + 1])
                        nc.sync.dma_start(
                            out=resT[dc * P:(dc + 1) * P, r0 + s0:r0 + s0 + SUB],
                            in_=rsb[:])

            # ---- phases 3+4: gathers with attention one pass behind ----
            Gts = {}
            for ps in range(NPASS):
                idxt, mskt, emapt = pre.pop(ps)
                if ps + 2 < NPASS:
                    pre[ps + 2] = load_gather_inputs(ps + 2)

                kv = kvp.tile([P, W], f32, tag="kv")
                nc.gpsimd.ap_gather(kv[:], table[:], idxt[:],
                                    channels=P, num_elems=NE, d=1, num_idxs=W)
                sbf = sbp.tile([P, W], bf16, tag="sbf")
                nc.vector.tensor_tensor_scan(
                    sbf[:], mskt[:], kv[:], 0.0,
                    op0=mybir.AluOpType.mult, op1=mybir.AluOpType.add)
                # local_scatter zero-fills and fully overwrites dst
                Gt = gtp.tile([P, CELLS], bf16, tag="g")
                nc.gpsimd.local_scatter(Gt[:], sbf[:], emapt[:],
                                        channels=P, num_elems=CELLS, num_idxs=W)
                Gts[ps] = Gt
                if ps >= 1:
                    # hoist the attention chunk's priority so the scheduler
                    # runs it inside the next gather's window instead of
                    # piling chunks after the last pass
                    with tc.high_priority(offset=400):
                        attn_chunk(ps - 1, Gts.pop(ps - 1))
            attn_chunk(NPASS - 1, Gts.pop(NPASS - 1))

    nc.compile()
    return nc


def prep_inputs(cfg: Cfg, x, adj_list, q_w, k_w, v_w, ego_scale, bias):
    """Host-side sharding/index prep.

    Returns (list of per-core input dicts, W). Sets cfg.W if unset.
    """
    import ml_dtypes
    N, D, H, K, NC = cfg.N, cfg.D, cfg.H, cfg.K, cfg.NC
    nr, C, NPASS, CELLS = cfg.nr, cfg.C, cfg.NPASS, cfg.CELLS

    src = np.asarray(adj_list[0], dtype=np.int64)
    dst = np.asarray(adj_list[1], dtype=np.int64)
    mk = np.asarray(adj_list[2]) != 0

    e_src, e_dst = src[mk], dst[mk]
    core = e_dst // nr
    lcell = e_dst % nr
    group = e_src // nr
    lsrc = (e_src % nr + 1).astype(np.int64)

    # balance cells across the 8 sub-passes to minimize the padded gather
    # width W = max over (core, group, pass) segment size
    cnt = np.zeros((NC, nr, 8), np.int32)
    np.add.at(cnt, (core, lcell, group), 1)
    perm = np.empty((NC, nr), np.int64)    # real cell -> column position
    for c in range(NC):
        tot = cnt[c].sum(1)
        order_c = np.argsort(-tot, kind="stable")
        loads = np.zeros((8, NPASS), np.int64)
        space = np.full(NPASS, CELLS)
        nxt = np.arange(NPASS) * CELLS
        for cell in order_c:
            v = cnt[c, cell][:, None]
            cand = (loads + v).max(0).astype(np.float64)
            cand[space == 0] = np.inf
            p = int(np.argmin(cand))
            loads[:, p] += cnt[c, cell]
            space[p] -= 1
            perm[c, cell] = nxt[p]
            nxt[p] += 1

    pos = perm[core, lcell]
    pss = pos // CELLS
    cell_in_ps = pos % CELLS

    key = ((core * 8 + group) * NPASS + pss) * 16384 + pos
    order = np.argsort(key, kind="stable")
    sc = core[order]
    sg = group[order]
    sp_ = pss[order]
    scell = cell_in_ps[order]
    ssrc = lsrc[order]

    seg = (sc * 8 + sg) * NPASS + sp_
    nseg = NC * 8 * NPASS
    seg_counts = np.bincount(seg, minlength=nseg)
    wmax = int(seg_counts.max())
    if cfg.W is None:
        cfg.W = max(16, -(-wmax // 16) * 16)
    W = cfg.W
    assert wmax <= W, f"W={W} too small for max segment {wmax}"
    seg_start = np.concatenate([[0], np.cumsum(seg_counts)])[:-1]
    pos = np.arange(len(order)) - seg_start[seg]

    same_prev = np.empty(len(order), dtype=bool)
    same_prev[0] = False
    same_prev[1:] = (seg[1:] == seg[:-1]) & (scell[1:] == scell[:-1])
    is_start = ~same_prev
    same_next = np.empty(len(order), dtype=bool)
    same_next[-1] = False
    same_next[:-1] = same_prev[1:]
    is_end = ~same_next

    idx8 = np.zeros((NC, NPASS, 8, W), np.int16)
    idx8[sc, sp_, sg, pos] = ssrc.astype(np.int16)
    msk8 = np.ones((NC, NPASS, 8, W), np.float32)
    msk8[sc[is_start], sp_[is_start], sg[is_start], pos[is_start]] = 0.0
    emap8 = np.full((NC, NPASS, 8, W), -1, np.int16)
    emap8[sc[is_end], sp_[is_end], sg[is_end], pos[is_end]] = \
        scell[is_end].astype(np.int16)

    # wrap idx streams: stream pos j -> (partition 16g + j%16, col j//16)
    idx_np = idx8.reshape(NC, NPASS, 8, W // 16, 16).transpose(0, 1, 2, 4, 3) \
        .reshape(NC, NPASS, P, W // 16)
    msk_np = np.repeat(msk8, 16, axis=2).astype(ml_dtypes.bfloat16)
    emap_np = np.repeat(emap8, 16, axis=2)

    ak = mk.reshape(N, K).sum(axis=1).astype(np.float32)
    inv_ak = (1.0 / (ak + 1e-6)).astype(np.float32)

    qkw_np = np.concatenate([np.asarray(q_w), np.asarray(k_w)],
                            axis=0).astype(np.float32)
    r16_np = (np.arange(P)[:, None] % 16 == np.arange(H)[None, :]) \
        .astype(ml_dtypes.bfloat16)

    in_maps = []
    for c in range(NC):
        xs = np.zeros((C, D), dtype=np.float32)
        xs[perm[c]] = np.asarray(x[c * nr:(c + 1) * nr], dtype=np.float32)
        xT = np.ascontiguousarray(xs.T).astype(ml_dtypes.bfloat16)

        ik = np.full((C,), 1e6, dtype=np.float32)
        ik[perm[c]] = inv_ak[c * nr:(c + 1) * nr]
        invk_np = np.broadcast_to(ik[None, :], (H, C)).copy()

        in_maps.append({
            "xT": xT,
            "qkw": qkw_np,
            "vw": np.asarray(v_w, dtype=np.float32),
            "ego": np.asarray(ego_scale, dtype=np.float32),
            "bias": np.asarray(bias, dtype=np.float32),
            "idx": idx_np[c],
            "msk": msk_np[c],
            "emap": emap_np[c],
            "invk": invk_np,
            "r16": r16_np,
        })
    return in_maps, W, perm


_CACHE = {}


def _get_compiled(cfg: Cfg):
    key = (cfg.N, cfg.D, cfg.H, cfg.K, cfg.NC, cfg.W)
    if key not in _CACHE:
        _CACHE[key] = build(cfg)
    return _CACHE[key]


def kernel(x, adj_list, q_w, k_w, v_w, ego_scale, bias, _trace=False):
    import concourse.bass_utils as bass_utils
    x = np.asarray(x)
    adj_list = np.asarray(adj_list)
    N, D = x.shape
    H = np.asarray(q_w).shape[0]
    K = adj_list.shape[1] // N
    cfg = Cfg(N=N, D=D, H=H, K=K, NC=8)

    dst = np.asarray(adj_list[1], dtype=np.int64)
    assert np.array_equal(dst, np.repeat(np.arange(N, dtype=np.int64), K)), \
        "kernel requires adj_list[1] grouped per target cell"

    in_maps, _, perm = prep_inputs(cfg, x, adj_list, q_w, k_w, v_w,
                                   ego_scale, bias)
    nc = _get_compiled(cfg)
    r = bass_utils.run_bass_kernel_spmd(nc, in_maps, core_ids=list(range(cfg.NC)),
                                        trace=_trace)
    out = np.concatenate(
        [r.results[c]["resT"].astype(np.float32).T[perm[c]]
         for c in range(cfg.NC)], axis=0)
    if _trace:
        return out, r
    return out

